# revision 3
# baseline (speedup 1.0000x reference)
"""DefocusBlur on 8 NeuronCores (Trainium2, Bass/Tile).

Depthwise 17x17 disk-blur of images [32,3,512,512] f32, reflect-101 pad.

Sharding: pure data parallel over batch — 4 images (12 planes) per core.

Per-core algorithm: the 2D conv is decomposed per kernel column j into a
1-D conv along H (as a PSUM-accumulated banded matmul, contraction over
128 padded input rows) with the W-shift j applied as a free-axis offset
into the W-padded input tile. The disk kernel is left-right symmetric
(kcol_j == kcol_{16-j}), so NPAIR mirror pairs are pre-summed on the
vector engine and share one matmul, balancing PE vs DVE. Inputs are
reflect-padded by 8 on the host so no edge logic runs on device.
Matmuls run as float32r (full PE rate at N=512, ~1e-4 rel err).
"""
import numpy as np

_RADIUS = 8
_B, _C, _H, _W = 32, 3, 512, 512
_NCORES = 8
_PLANES = (_B // _NCORES) * _C
_M = 112
_KIN = _M + 2 * _RADIUS
_NBLK = 5
_HP = _H + 2 * _RADIUS
_WP = _W + 2 * _RADIUS

NPAIR = 4  # pairs pre-summed on DVE; groups = 17 - NPAIR


def _disk_kernel():
    L = np.arange(-8, 9)
    X, Y = np.meshgrid(L, L)
    disk = ((X ** 2 + Y ** 2) <= _RADIUS ** 2).astype(np.float32)
    disk /= disk.sum()
    x = np.arange(3, dtype=np.float32) - 1
    g = np.exp(-(x ** 2) / (2.0 * 0.5 ** 2))
    g /= g.sum()
    k2 = np.outer(g, g).astype(np.float32)
    p = np.pad(disk, 1, mode="reflect")
    out = np.zeros_like(disk)
    for i in range(3):
        for j in range(3):
            out += k2[i, j] * p[i : i + 17, j : j + 17]
    return out


def _groups():
    """Returns list of (cols, kcol_index): cols = list of W-shifts sharing
    banded weight kcol_index."""
    gs = []
    for j in range(NPAIR):
        gs.append(([j, 16 - j], j))
    for j in range(NPAIR, 17 - NPAIR):
        gs.append(([j], j))
    return gs


def _banded_weights():
    k2d = _disk_kernel()
    ws = []
    for _, j in _groups():
        B = np.zeros((_KIN, _M), np.float32)
        for c in range(_M):
            B[c : c + 17, c] = k2d[:, j]
        ws.append(B)
    return np.ascontiguousarray(np.concatenate(ws, axis=1))


_NC_CACHE = []


def _build_program():
    import concourse.bacc as bacc
    import concourse.mybir as mybir
    import concourse.tile as tile

    f32 = mybir.dt.float32
    f32r = mybir.dt.float32r
    gs = _groups()
    ng = len(gs)

    nc = bacc.Bacc("TRN2", target_bir_lowering=False, debug=False)
    x_d = nc.dram_tensor("x", [_PLANES, _HP, _WP], f32r, kind="ExternalInput")
    w_d = nc.dram_tensor("w", [_KIN, ng * _M], f32r, kind="ExternalInput")
    o_d = nc.dram_tensor("o", [_PLANES, _H, _W], f32, kind="ExternalOutput")

    with tile.TileContext(nc) as tc:
        with (
            tc.tile_pool(name="wpool", bufs=1) as wpool,
            tc.tile_pool(name="inp", bufs=3) as inp,
            tc.tile_pool(name="spool", bufs=2) as spool,
            tc.tile_pool(name="outp", bufs=3) as outp,
            tc.tile_pool(name="ps", bufs=3, space="PSUM") as psp,
        ):
            wt = wpool.tile([_KIN, ng * _M], f32r)
            nc.sync.dma_start(wt[:], w_d[:])
            for p in range(_PLANES):
                for b in range(_NBLK):
                    mb = min(_M, _H - b * _M)
                    kb = mb + 2 * _RADIUS
                    xt = inp.tile([_KIN, _WP], f32r, tag="xt")
                    nc.sync.dma_start(
                        xt[:kb, :], x_d[p, b * _M : b * _M + kb, :]
                    )
                    # pair sums on DVE
                    stiles = []
                    for gi, (cols, _) in enumerate(gs):
                        if len(cols) == 2:
                            st = spool.tile([_KIN, _W], f32r, tag=f"s{gi}")
                            j0, j1 = cols
                            nc.vector.tensor_add(
                                st[:kb, :],
                                xt[:kb, j0 : j0 + _W],
                                xt[:kb, j1 : j1 + _W],
                            )
                            stiles.append(st)
                        else:
                            stiles.append(None)
                    ps = psp.tile([_M, _W], f32, tag="ps")
                    # interleave pair-matmuls among singles so PE never
                    # stalls on a DVE pair-sum (sim-tuned pattern)
                    singles = [gi for gi, (c, _) in enumerate(gs) if len(c) == 1]
                    pairs = [gi for gi, (c, _) in enumerate(gs) if len(c) == 2]
                    order = (
                        singles[:1] + pairs[:1] + singles[1:3] + pairs[1:2]
                        + singles[3:5] + pairs[2:3] + singles[5:9] + pairs[3:]
                    )
                    for mi, gi in enumerate(order):
                        cols, _ = gs[gi]
                        if len(cols) == 1:
                            rhs = xt[:kb, cols[0] : cols[0] + _W]
                        else:
                            rhs = stiles[gi][:kb, :]
                        nc.tensor.matmul(
                            ps[:mb, :],
                            wt[:kb, gi * _M : gi * _M + mb],
                            rhs,
                            start=(mi == 0),
                            stop=(mi == ng - 1),
                        )
                    ot = outp.tile([_M, _W], f32, tag="ot")
                    nc.scalar.copy(ot[:mb, :], ps[:mb, :])
                    nc.sync.dma_start(
                        o_d[p, b * _M : b * _M + mb, :], ot[:mb, :]
                    )
    nc.compile()
    return nc


def _get_program():
    if not _NC_CACHE:
        _NC_CACHE.append(_build_program())
    return _NC_CACHE[0]


def kernel(images: np.ndarray) -> np.ndarray:
    from concourse.bass_utils import run_bass_kernel_spmd

    images = np.asarray(images, dtype=np.float32)
    padded = np.pad(
        images, ((0, 0), (0, 0), (_RADIUS, _RADIUS), (_RADIUS, _RADIUS)),
        mode="reflect",
    )
    shards = np.ascontiguousarray(padded.reshape(_NCORES, _PLANES, _HP, _WP))
    w = _banded_weights()
    nc = _get_program()
    in_maps = [{"x": shards[c], "w": w} for c in range(_NCORES)]
    res = run_bass_kernel_spmd(nc, in_maps, list(range(_NCORES)))
    out = np.stack([res.results[c]["o"] for c in range(_NCORES)], axis=0)
    return np.ascontiguousarray(out.reshape(_B, _C, _H, _W).astype(np.float32))
